# revision 2
# baseline (speedup 1.0000x reference)
"""Self-contained Trainium2 Bass kernel for the 2-layer Llama3 model.

Sharding: token-sharded over 8 cores. Core c owns token blocks {c, 15-c}
(128 tokens each) of each batch -> 512 tokens/core, causally balanced.
Per layer each core computes q/k/v for its own tokens, AllGathers k/v
(bf16, 512KB/rank), runs attention for its own query rows over the full
sequence, then FFN and the vocab head token-locally with full bf16
weights. Communication total: 2 small AllGathers.

Device layouts: activations transposed [feature, token] in 128-partition
chunks; scores computed transposed [sk, sq]; softmax denominator via a
ones-augmented column on v; RoPE via a +-1 rotation matrix on the PE.
SPMD-uniform attention: low block always 8 k-chunks, high block always
16; per-core causality lives in mask *data* (ones/tri/zeros inputs).
"""
from contextlib import ExitStack

import numpy as np
import ml_dtypes

import concourse.bass as bass
from concourse.bacc import Bacc
import concourse.mybir as mybir
import concourse.tile as tile
from concourse.bass_utils import run_bass_kernel_spmd
from concourse.masks import make_identity

BF16 = ml_dtypes.bfloat16
F32 = mybir.dt.float32
BF = mybir.dt.bfloat16
I32 = mybir.dt.int32

V, D, H, KVH, HD, F, L, B, S = 32000, 1024, 16, 4, 64, 4096, 2, 2, 2048
ROPE_BASE = 500000.0
EPS = 1e-5
SCALE = 1.0 / HD ** 0.5
C = 8            # cores
NB = 16          # 128-token blocks per batch
BS = 128         # block size
OWN = 512        # own tokens per core (2 blocks x 2 batches)
NVT = (V + 511) // 512   # 63 head v-tiles (last is 256 wide)

Exp = mybir.ActivationFunctionType.Exp
Silu = mybir.ActivationFunctionType.Silu
Sqrt = mybir.ActivationFunctionType.Sqrt
Square = mybir.ActivationFunctionType.Square
MULT = mybir.AluOpType.mult
ADD = mybir.AluOpType.add


def own_blocks(c):
    return [c, NB - 1 - c]


def _colseg(b, t):
    """Own-token column range for (batch b, tile t in {0=low,1=high})."""
    return slice(256 * b + 128 * t, 256 * b + 128 * (t + 1))


# ---------------------------------------------------------------- device ---

def build_nc():
    nc = Bacc()

    idx = nc.dram_tensor("idx", [128, 4], I32, kind="ExternalInput")
    emb = nc.dram_tensor("emb", [V, D], F32, kind="ExternalInput")
    cosT = nc.dram_tensor("cosT", [128, OWN], F32, kind="ExternalInput")
    sinT = nc.dram_tensor("sinT", [128, OWN], F32, kind="ExternalInput")
    rmat = nc.dram_tensor("rmat", [128, 128], BF, kind="ExternalInput")
    masks = nc.dram_tensor("masks", [16, 128, 512], BF, kind="ExternalInput")
    wq = nc.dram_tensor("wq", [L, D, D], BF, kind="ExternalInput")
    wk = nc.dram_tensor("wk", [L, D, KVH * HD], BF, kind="ExternalInput")
    wv = nc.dram_tensor("wv", [L, D, KVH * HD], BF, kind="ExternalInput")
    wo = nc.dram_tensor("wo", [L, D, D], BF, kind="ExternalInput")
    wg = nc.dram_tensor("wg", [L, D, F], BF, kind="ExternalInput")
    wu = nc.dram_tensor("wu", [L, D, F], BF, kind="ExternalInput")
    wd = nc.dram_tensor("wd", [L, F, D], BF, kind="ExternalInput")
    wout = nc.dram_tensor("wout", [D, V], BF, kind="ExternalInput")
    logits = nc.dram_tensor("logits", [OWN, V], F32, kind="ExternalOutput")

    kvs = [nc.dram_tensor(f"kvs{l}", [B, 2, 256, 256], BF) for l in range(L)]
    kvr = [nc.dram_tensor(f"kvr{l}", [C, B, 2, 256, 256], BF,
                          addr_space="Shared") for l in range(L)]

    with tile.TileContext(nc) as tc, ExitStack() as st:
        const = st.enter_context(tc.tile_pool(name="const", bufs=1))
        resid = st.enter_context(tc.tile_pool(name="resid", bufs=1))
        npool = st.enter_context(tc.tile_pool(name="npool", bufs=1))
        qpool = st.enter_context(tc.tile_pool(name="qpool", bufs=1))
        apool = st.enter_context(tc.tile_pool(name="apool", bufs=1))
        hpool = st.enter_context(tc.tile_pool(name="hpool", bufs=1))
        sb = st.enter_context(tc.tile_pool(name="sb", bufs=2))
        wbig = st.enter_context(tc.tile_pool(name="wbig", bufs=2))
        psA = st.enter_context(tc.tile_pool(name="psA", bufs=2, space="PSUM"))
        psB = st.enter_context(tc.tile_pool(name="psB", bufs=2, space="PSUM"))
        psM = st.enter_context(tc.tile_pool(name="psM", bufs=3, space="PSUM"))

        # constants
        ident = const.tile([128, 128], F32, tag="ident")
        make_identity(nc, ident)
        ones_col = const.tile([128, 1], BF, tag="ones_col")
        nc.any.memset(ones_col[:], 1.0)
        ones_row = const.tile([1, 128], BF, tag="ones_row")
        nc.any.memset(ones_row[:], 1.0)
        eps_t = const.tile([1, 1], F32, tag="eps")
        nc.any.memset(eps_t[:], EPS)
        t_rmat = const.tile([128, 128], BF, tag="rmat")
        nc.sync.dma_start(out=t_rmat[:], in_=rmat[:])
        t_cos = const.tile([128, OWN], F32, tag="cos")
        nc.sync.dma_start(out=t_cos[:], in_=cosT[:])
        t_sin = const.tile([128, OWN], F32, tag="sin")
        nc.sync.dma_start(out=t_sin[:], in_=sinT[:])
        t_masks = [const.tile([128, 512], BF, tag=f"mask{m}", name=f"mask{m}") for m in range(16)]
        for m in range(16):
            nc.sync.dma_start(out=t_masks[m][:], in_=masks[m])

        # residual stream xT: 8 chunks [128, OWN] f32, resident
        x = [resid.tile([128, OWN], F32, tag=f"x{k}", name=f"x{k}") for k in range(8)]

        # ---- embedding gather + transpose ----
        t_idx = const.tile([128, 4], I32, tag="idx")
        nc.sync.dma_start(out=t_idx[:], in_=idx[:])
        for sbk in range(4):
            g = sb.tile([128, D], F32, tag="embg")
            nc.gpsimd.indirect_dma_start(
                out=g[:], out_offset=None, in_=emb[:],
                in_offset=bass.IndirectOffsetOnAxis(ap=t_idx[:, sbk:sbk + 1], axis=0))
            for k in range(8):
                pt = psM.tile([128, 128], F32, tag="pmm")
                nc.tensor.transpose(out=pt[:], in_=g[:, 128 * k:128 * (k + 1)],
                                    identity=ident[:])
                nc.vector.tensor_copy(out=x[k][:, 128 * sbk:128 * (sbk + 1)],
                                      in_=pt[:])

        def rmsnorm():
            """x -> n bf16 chunks (npool tags n0..n7, reused per call)."""
            ssq = psB.tile([1, OWN], F32, tag="psB")
            for k in range(8):
                x2 = sb.tile([128, OWN], BF, tag="x2")
                nc.scalar.activation(out=x2[:], in_=x[k][:], func=Square)
                nc.tensor.matmul(out=ssq[:], lhsT=ones_col[:], rhs=x2[:],
                                 start=(k == 0), stop=(k == 7))
            rms = sb.tile([1, OWN], F32, tag="rms", bufs=1)
            nc.scalar.activation(out=rms[:], in_=ssq[:], func=Sqrt,
                                 scale=1.0 / D, bias=eps_t[:])
            inv = sb.tile([1, OWN], F32, tag="inv", bufs=1)
            nc.vector.reciprocal(out=inv[:], in_=rms[:])
            inv_bf = sb.tile([1, OWN], BF, tag="invbf", bufs=1)
            nc.vector.tensor_copy(out=inv_bf[:], in_=inv[:])
            binv = psB.tile([128, OWN], F32, tag="psB")
            nc.tensor.matmul(out=binv[:], lhsT=ones_row[:], rhs=inv_bf[:],
                             start=True, stop=True)
            n = [npool.tile([128, OWN], BF, tag=f"n{k}", name=f"n{k}") for k in range(8)]
            for k in range(8):
                nc.vector.tensor_tensor(out=n[k][:], in0=x[k][:], in1=binv[:],
                                        op=MULT)
            return n

        def rope(pm, dst_tag_pool_tile):
            """pm: psum [128, OWN] pre-rope -> bf16 tile (given) with rope."""
            y = dst_tag_pool_tile
            yr = sb.tile([128, OWN], BF, tag="prerope")
            nc.vector.tensor_copy(out=yr[:], in_=pm[:])
            rot = psA.tile([128, OWN], F32, tag="psA")
            nc.tensor.matmul(out=rot[:], lhsT=t_rmat[:], rhs=yr[:],
                             start=True, stop=True)
            tmp1 = sb.tile([128, OWN], F32, tag="ropet1", bufs=1)
            nc.vector.tensor_tensor(out=tmp1[:], in0=yr[:], in1=t_cos[:], op=MULT)
            tmp2 = sb.tile([128, OWN], F32, tag="ropet2", bufs=1)
            nc.vector.tensor_tensor(out=tmp2[:], in0=rot[:], in1=t_sin[:], op=MULT)
            nc.vector.tensor_tensor(out=y[:], in0=tmp1[:], in1=tmp2[:], op=ADD)
            return y

        def proj_blocked(n, w_ap, mo_count, out_tiles, do_rope, wtag):
            """out_tiles[mo] [128, OWN] = (W.T @ n) chunks, blocked weight fetch.
            w_ap: [D, mo_count*128]. Fetches [128, 512] weight tiles."""
            mo_per = 4 if mo_count >= 4 else mo_count
            for mb in range(mo_count // mo_per):
                wts = []
                for k in range(8):
                    wt = wbig.tile([128, 128 * mo_per], BF, tag=f"wbig{k}",
                                   name=f"wt{k}")
                    nc.sync.dma_start(
                        out=wt[:],
                        in_=w_ap[128 * k:128 * (k + 1),
                                 128 * mo_per * mb:128 * mo_per * (mb + 1)])
                    wts.append(wt)
                for ms in range(mo_per):
                    mo = mo_per * mb + ms
                    pm = psM.tile([128, OWN], F32, tag="pmm")
                    for k in range(8):
                        nc.tensor.matmul(
                            out=pm[:], lhsT=wts[k][:, 128 * ms:128 * (ms + 1)],
                            rhs=n[k][:], start=(k == 0), stop=(k == 7))
                    if do_rope:
                        rope(pm, out_tiles[mo])
                    else:
                        nc.vector.tensor_copy(out=out_tiles[mo][:], in_=pm[:])

        for l in range(L):
            n = rmsnorm()
            qr = [qpool.tile([128, OWN], BF, tag=f"qr{mo}", name=f"qr{mo}") for mo in range(8)]
            proj_blocked(n, wq[l], 8, qr, True, "pwq")
            kr = [sb.tile([128, OWN], BF, tag=f"kr{mo}", name=f"kr{mo}") for mo in range(2)]
            proj_blocked(n, wk[l], 2, kr, True, "pwk")
            # v natural [own tok, 256]; lhsT = n col-slices (stationary)
            wvt = []
            for k in range(8):
                wt = wbig.tile([128, 256], BF, tag=f"wbig{k}", name=f"wvt{k}")
                nc.sync.dma_start(out=wt[:], in_=wv[l, 128 * k:128 * (k + 1), :])
                wvt.append(wt)
            for t in range(4):
                pv = psM.tile([128, 256], F32, tag="pmm")
                for k in range(8):
                    nc.tensor.matmul(out=pv[:],
                                     lhsT=n[k][:, 128 * t:128 * (t + 1)],
                                     rhs=wvt[k][:], start=(k == 0), stop=(k == 7))
                vt = sb.tile([128, 256], BF, tag="vnat")
                nc.vector.tensor_copy(out=vt[:], in_=pv[:])
                nc.sync.dma_start(
                    out=kvs[l][t // 2, 1, 128 * (t % 2):128 * (t % 2 + 1), :],
                    in_=vt[:])
            for b in range(B):
                for mo in range(2):
                    nc.sync.dma_start(
                        out=kvs[l][b, 0, 128 * mo:128 * (mo + 1), :],
                        in_=kr[mo][:, 256 * b:256 * (b + 1)])
            nc.gpsimd.collective_compute(
                "AllGather", mybir.AluOpType.bypass,
                replica_groups=[list(range(C))],
                ins=[kvs[l][:]], outs=[kvr[l][:]])

            # ---- attention (per batch: assemble k/v, run units) ----
            casm = [apool.tile([128, OWN], BF, tag=f"casm{k}", name=f"casm{k}") for k in range(8)]
            for b in range(B):
                kT = [apool.tile([64, S], BF, tag=f"kt{g}", name=f"kt{g}") for g in range(KVH)]
                for g in range(KVH):
                    src = kvr[l][:, b, 0, 64 * g:64 * (g + 1), :]   # [C,64,256]
                    # low blocks of ranks 0..7 -> cols 0:1024, one DMA
                    nc.sync.dma_start(
                        out=kT[g][:, 0:1024].rearrange("p (r c) -> p r c", r=C),
                        in_=src[:, :, 0:128].transpose([1, 0, 2]))
                    # high block of rank r -> block 15-r
                    for r in range(C):
                        nc.sync.dma_start(
                            out=kT[g][:, 128 * (NB - 1 - r):128 * (NB - r)],
                            in_=src[r, :, 128:256])
                v4 = [apool.tile([128, 260], BF, tag=f"v4{j}", name=f"v4{j}") for j in range(NB)]
                for j in range(NB):
                    r, i = (j, 0) if j < C else (NB - 1 - j, 1)
                    dst = v4[j][:].rearrange("p (g c) -> p g c", g=4)
                    nc.sync.dma_start(
                        out=dst[:, :, 0:64],
                        in_=kvr[l][r, b, 1, 128 * i:128 * (i + 1), :]
                            .rearrange("p (g c) -> p g c", g=4))
                    nc.any.memset(dst[:, :, 64:65], 1.0)

                for g in range(KVH):
                    for t in range(2):          # t=0: low block, t=1: high
                        qp = sb.tile([64, 512], BF, tag="qpack")
                        for i in range(4):
                            h = 4 * g + i
                            mo, ro = divmod(h, 2)
                            nc.vector.tensor_copy(
                                out=qp[:, 128 * i:128 * (i + 1)],
                                in_=qr[mo][64 * ro:64 * (ro + 1), _colseg(b, t)])
                        ctx = psB.tile([65, 512], F32, tag="psB")
                        nj = 8 if t == 0 else 16
                        for j in range(nj):
                            sc = psA.tile([128, 512], F32, tag="psA")
                            nc.tensor.matmul(
                                out=sc[:], lhsT=kT[g][:, 128 * j:128 * (j + 1)],
                                rhs=qp[:], start=True, stop=True)
                            ex = sb.tile([128, 512], BF, tag="exp")
                            nc.scalar.activation(out=ex[:], in_=sc[:], func=Exp)
                            if t == 0 or j >= 8:
                                exm = sb.tile([128, 512], BF, tag="expm")
                                m = t_masks[j if t == 0 else j - 8 + 8]
                                nc.vector.tensor_tensor(out=exm[:], in0=ex[:],
                                                        in1=m[:], op=MULT)
                                ex = exm
                            nc.tensor.matmul(
                                out=ctx[:], lhsT=v4[j][:, 65 * g:65 * (g + 1)],
                                rhs=ex[:], start=(j == 0), stop=(j == nj - 1))
                        rec = sb.tile([1, 512], F32, tag="rec")
                        nc.vector.reciprocal(out=rec[:], in_=ctx[64:65, :])
                        rec_bf = sb.tile([1, 512], BF, tag="recbf")
                        nc.vector.tensor_copy(out=rec_bf[:], in_=rec[:])
                        brec = psA.tile([64, 512], F32, tag="psA")
                        nc.tensor.matmul(out=brec[:], lhsT=ones_row[:1, 0:64],
                                         rhs=rec_bf[:], start=True, stop=True)
                        brec_s = sb.tile([64, 512], BF, tag="brecs")
                        nc.vector.tensor_copy(out=brec_s[:], in_=brec[:])
                        for i in range(4):
                            h = 4 * g + i
                            mo, ro = divmod(h, 2)
                            nc.vector.tensor_tensor(
                                out=casm[mo][64 * ro:64 * (ro + 1), _colseg(b, t)],
                                in0=ctx[0:64, 128 * i:128 * (i + 1)],
                                in1=brec_s[:, 128 * i:128 * (i + 1)], op=MULT)

            # ---- wo + residual ----
            for mb in range(2):
                wts = []
                for k in range(8):
                    wt = wbig.tile([128, 512], BF, tag=f"wbig{k}",
                                   name=f"wot{k}")
                    nc.sync.dma_start(
                        out=wt[:],
                        in_=wo[l, 128 * k:128 * (k + 1), 512 * mb:512 * (mb + 1)])
                    wts.append(wt)
                for ms in range(4):
                    mo = 4 * mb + ms
                    pm = psM.tile([128, OWN], F32, tag="pmm")
                    for k in range(8):
                        nc.tensor.matmul(
                            out=pm[:], lhsT=wts[k][:, 128 * ms:128 * (ms + 1)],
                            rhs=casm[k][:], start=(k == 0), stop=(k == 7))
                    nc.vector.tensor_tensor(out=x[mo][:], in0=x[mo][:],
                                            in1=pm[:], op=ADD)

            # ---- FFN ----
            n2 = rmsnorm()
            ht = [hpool.tile([128, OWN], BF, tag=f"h{mo}", name=f"h{mo}") for mo in range(32)]
            for mb in range(8):
                wgt, wut = [], []
                for k in range(8):
                    a = wbig.tile([128, 512], BF, tag=f"wbig{k}", name=f"wgt{k}")
                    nc.sync.dma_start(
                        out=a[:],
                        in_=wg[l, 128 * k:128 * (k + 1), 512 * mb:512 * (mb + 1)])
                    wgt.append(a)
                    u = wbig.tile([128, 512], BF, tag=f"wu{k}", name=f"wut{k}")
                    nc.sync.dma_start(
                        out=u[:],
                        in_=wu[l, 128 * k:128 * (k + 1), 512 * mb:512 * (mb + 1)])
                    wut.append(u)
                for ms in range(4):
                    mo = 4 * mb + ms
                    pg = psM.tile([128, OWN], F32, tag="pmm")
                    for k in range(8):
                        nc.tensor.matmul(
                            out=pg[:], lhsT=wgt[k][:, 128 * ms:128 * (ms + 1)],
                            rhs=n2[k][:], start=(k == 0), stop=(k == 7))
                    gs = sb.tile([128, OWN], BF, tag="gsilu")
                    nc.scalar.activation(out=gs[:], in_=pg[:], func=Silu)
                    pu = psM.tile([128, OWN], F32, tag="pmm")
                    for k in range(8):
                        nc.tensor.matmul(
                            out=pu[:], lhsT=wut[k][:, 128 * ms:128 * (ms + 1)],
                            rhs=n2[k][:], start=(k == 0), stop=(k == 7))
                    nc.vector.tensor_tensor(out=ht[mo][:], in0=pu[:], in1=gs[:],
                                            op=MULT)
            for mo in range(8):
                pd = psM.tile([128, OWN], F32, tag="pmm")
                for k in range(32):
                    wt = wbig.tile([128, 128], BF, tag="wsm", bufs=4,
                                   name="wdt")
                    nc.sync.dma_start(
                        out=wt[:],
                        in_=wd[l, 128 * k:128 * (k + 1), 128 * mo:128 * (mo + 1)])
                    nc.tensor.matmul(out=pd[:], lhsT=wt[:], rhs=ht[k][:],
                                     start=(k == 0), stop=(k == 31))
                nc.vector.tensor_tensor(out=x[mo][:], in0=x[mo][:],
                                        in1=pd[:], op=ADD)

        # ---- final norm + head ----
        nf = rmsnorm()
        for vt in range(NVT):
            vw = min(512, V - 512 * vt)
            wts = []
            for k in range(8):
                wt = wbig.tile([128, 512], BF, tag=f"wbig{k}", name=f"wht{k}")
                nc.sync.dma_start(
                    out=wt[:, :vw],
                    in_=wout[128 * k:128 * (k + 1), 512 * vt:512 * vt + vw])
                wts.append(wt)
            for t in range(4):
                ph = psM.tile([128, 512], F32, tag="pmm")
                for k in range(8):
                    nc.tensor.matmul(out=ph[:, :vw],
                                     lhsT=nf[k][:, 128 * t:128 * (t + 1)],
                                     rhs=wts[k][:, :vw],
                                     start=(k == 0), stop=(k == 7))
                ot = sb.tile([128, 512], F32, tag="hout")
                nc.vector.tensor_copy(out=ot[:, :vw], in_=ph[:, :vw])
                nc.sync.dma_start(
                    out=logits[128 * t:128 * (t + 1), 512 * vt:512 * vt + vw],
                    in_=ot[:, :vw])

    return nc


# ------------------------------------------------------------------ host ---

_NC_CACHE = {}


def _get_nc():
    if "nc" not in _NC_CACHE:
        nc = build_nc()
        nc.finalize()
        _NC_CACHE["nc"] = nc
    return _NC_CACHE["nc"]


def _host_prep(inputs):
    inv_freq = 1.0 / ROPE_BASE ** (np.arange(0, HD, 2, dtype=np.float32) / HD)
    t = np.arange(S, dtype=np.float32)
    freqs = t[:, None] * inv_freq[None, :]
    ang = np.concatenate([freqs, freqs], axis=-1)       # [S, 64]
    cos_full, sin_full = np.cos(ang), np.sin(ang)
    cosT2 = np.empty((128, S), np.float32)
    sinT2 = np.empty((128, S), np.float32)
    for p in range(128):
        d = p % 64
        cosT2[p] = cos_full[:, d]
        sinT2[p] = sin_full[:, d] * (-1.0 if d < 32 else 1.0)

    R = np.zeros((128, 128), np.float32)
    for blk in range(2):
        o = blk * 64
        for j in range(32):
            R[o + 32 + j, o + j] = 1.0
            R[o + j, o + 32 + j] = 1.0

    naw = np.asarray(inputs["norm_attn_w"], np.float32)
    nfw = np.asarray(inputs["norm_ff_w"], np.float32)
    prep = {
        "emb": np.ascontiguousarray(np.asarray(inputs["token_emb"], np.float32)),
        "rmat": np.ascontiguousarray(R.astype(BF16)),
        "wq": np.ascontiguousarray(
            (np.asarray(inputs["wq"], np.float32) * naw[:, :, None] * SCALE).astype(BF16)),
        "wk": np.ascontiguousarray(
            (np.asarray(inputs["wk"], np.float32) * naw[:, :, None]).astype(BF16)),
        "wv": np.ascontiguousarray(
            (np.asarray(inputs["wv"], np.float32) * naw[:, :, None]).astype(BF16)),
        "wo": np.ascontiguousarray(np.asarray(inputs["wo"], np.float32).astype(BF16)),
        "wg": np.ascontiguousarray(
            (np.asarray(inputs["w_gate"], np.float32) * nfw[:, :, None]).astype(BF16)),
        "wu": np.ascontiguousarray(
            (np.asarray(inputs["w_up"], np.float32) * nfw[:, :, None]).astype(BF16)),
        "wd": np.ascontiguousarray(np.asarray(inputs["w_down"], np.float32).astype(BF16)),
        "wout": np.ascontiguousarray(
            (np.asarray(inputs["w_out"], np.float32)
             * np.asarray(inputs["norm_final_w"], np.float32)[:, None]).astype(BF16)),
    }

    idx_full = np.asarray(inputs["in_idx"], np.int64)
    tri = (np.arange(128)[:, None] <= np.arange(128)[None, :]).astype(np.float32)
    tri4 = np.tile(tri, (1, 4))
    in_maps = []
    for c in range(C):
        blks = own_blocks(c)
        pos = np.concatenate([np.arange(bl * BS, (bl + 1) * BS) for bl in blks])
        own_idx = np.empty((128, 4), np.int32)
        for sbk in range(4):
            b, t = divmod(sbk, 2)
            own_idx[:, sbk] = idx_full[b, blks[t] * BS:(blks[t] + 1) * BS]
        cosT = np.ascontiguousarray(
            np.concatenate([cosT2[:, pos], cosT2[:, pos]], axis=1))
        sinT = np.ascontiguousarray(
            np.concatenate([sinT2[:, pos], sinT2[:, pos]], axis=1))
        # masks [16, 128, 512]: 0..7 low block (blk c) j=0..7,
        #                       8..15 high block (blk 15-c) j=8..15
        mk = np.zeros((16, 128, 512), np.float32)
        for t, blk in enumerate(blks):
            for jj in range(8):
                j = jj if t == 0 else jj + 8
                if j < blk:
                    mk[8 * t + jj] = 1.0
                elif j == blk:
                    mk[8 * t + jj] = tri4
        in_maps.append({
            "idx": own_idx,
            "cosT": cosT,
            "sinT": sinT,
            "masks": np.ascontiguousarray(mk.astype(BF16)),
            **prep,
        })
    return in_maps


def _assemble(results):
    out = np.empty((B, S, V), np.float32)
    for c in range(C):
        lg = np.asarray(results[c]["logits"])
        blks = own_blocks(c)
        for b in range(B):
            for t, blk in enumerate(blks):
                out[b, blk * BS:(blk + 1) * BS] = \
                    lg[256 * b + 128 * t:256 * b + 128 * (t + 1)]
    return out


def run(inputs, trace=False, trace_cores=None):
    nc = _get_nc()
    in_maps = _host_prep(inputs)
    res = run_bass_kernel_spmd(nc, in_maps, list(range(C)), trace=trace,
                               trace_cores=trace_cores)
    return _assemble(res.results), res


def kernel(**inputs):
    out, _ = run(inputs)
    return out



# revision 6
# speedup vs baseline: 1.0049x; 1.0049x over previous
"""Self-contained Trainium2 Bass kernel for the 2-layer Llama3 model.

Sharding: token-sharded compute over 8 cores (core c owns token blocks
{c, 15-c} of each batch, 512 tokens/core), with *minimal host->device
input bytes*: every weight is shipped exactly once, row-sharded by its
contraction dim (1/8 per core), and AllGathered on-device over the
intra-chip links, overlapped with compute. Embedding rows are gathered
on host (2MB/core instead of the 131MB fp32 table). The vocab head is
vocab-sharded: each core keeps its [D, V/8] slice of w_out, final
hidden states are AllGathered (8MB), and each core computes logits for
ALL tokens x its vocab slice, emitted as fp16 (host casts to f32).

Per-core input ~20MB (vs ~257MB replicated), output 32MB fp16.

Device layouts: activations transposed [feature, token] in 128-part
chunks; scores computed transposed [sk, sq]; softmax denominator via a
ones-augmented column on v; RoPE via a +-1 rotation matrix on the PE.
SPMD-uniform program: per-core causality lives in mask *data*.
"""
from contextlib import ExitStack

import numpy as np
import ml_dtypes

import concourse.bass as bass
from concourse.bacc import Bacc
import concourse.mybir as mybir
import concourse.tile as tile
from concourse.bass_utils import run_bass_kernel_spmd

BF16 = ml_dtypes.bfloat16
F32 = mybir.dt.float32
F16 = mybir.dt.float16
BF = mybir.dt.bfloat16

V, D, H, KVH, HD, F, L, B, S = 32000, 1024, 16, 4, 64, 4096, 2, 2, 2048
ROPE_BASE = 500000.0
EPS = 1e-5
SCALE = 1.0 / HD ** 0.5
C = 8            # cores
NB = 16          # 128-token blocks per batch
BS = 128         # block size
OWN = 512        # own tokens per core (2 blocks x 2 batches)
VS = V // C      # vocab shard per core (4000)
NVT = (VS + 511) // 512   # 8 vocab tiles (last is 416 wide)

Exp = mybir.ActivationFunctionType.Exp
Silu = mybir.ActivationFunctionType.Silu
Sqrt = mybir.ActivationFunctionType.Sqrt
Square = mybir.ActivationFunctionType.Square
MULT = mybir.AluOpType.mult
ADD = mybir.AluOpType.add


def own_blocks(c):
    return [c, NB - 1 - c]


def _colseg(b, t):
    """Own-token column range for (batch b, tile t in {0=low,1=high})."""
    return slice(256 * b + 128 * t, 256 * b + 128 * (t + 1))


# ---------------------------------------------------------------- device ---

def build_nc():
    nc = Bacc()

    x0 = nc.dram_tensor("x0", [8, 128, OWN], F32, kind="ExternalInput")
    cosT = nc.dram_tensor("cosT", [128, OWN], F32, kind="ExternalInput")
    sinT = nc.dram_tensor("sinT", [128, OWN], F32, kind="ExternalInput")
    rmat = nc.dram_tensor("rmat", [128, 128], BF, kind="ExternalInput")
    masks = nc.dram_tensor("masks", [16, 128, 512], BF, kind="ExternalInput")
    # weight shards, row-sharded by contraction dim (1/8 per core):
    # qkvo_sh cols: wq 0:1024 | wk 1024:1280 | wv 1280:1536 | wo 1536:2560
    qkvo_sh = nc.dram_tensor("qkvo_sh", [L, 128, 2560], BF, kind="ExternalInput")
    # gu_sh cols: 8 groups of [wg 512 | wu 512]
    gu_sh = nc.dram_tensor("gu_sh", [L, 128, 8192], BF, kind="ExternalInput")
    # wd_sh cols: 4 groups of 1024 (F-chunks 4c..4c+3, each [128, D])
    wd_sh = nc.dram_tensor("wd_sh", [L, 128, 4096], BF, kind="ExternalInput")
    woutc = nc.dram_tensor("woutc", [D, VS], BF, kind="ExternalInput")
    logits = nc.dram_tensor("logits", [C * OWN, VS], F16, kind="ExternalOutput")

    # collective staging (internal) and gathered (Shared) buffers
    qkvo_st = nc.dram_tensor("qkvo_st", [L, 128, 2560], BF)
    gu_st = nc.dram_tensor("gu_st", [L, 128, 8192], BF)
    wd_st = nc.dram_tensor("wd_st", [L, 128, 4096], BF)
    nf_st = nc.dram_tensor("nf_st", [128, 8 * OWN], BF)
    qkvo_g = [nc.dram_tensor(f"qkvog{l}", [C, 128, 2560], BF,
                             addr_space="Shared") for l in range(L)]
    gu_g = [nc.dram_tensor(f"gug{l}", [C, 128, 8192], BF,
                           addr_space="Shared") for l in range(L)]
    wd_g = [nc.dram_tensor(f"wdg{l}", [C, 128, 4096], BF,
                           addr_space="Shared") for l in range(L)]
    nf_g = nc.dram_tensor("nfg", [C, 128, 8 * OWN], BF, addr_space="Shared")
    kvs = [nc.dram_tensor(f"kvs{l}", [B, 2, 256, 256], BF) for l in range(L)]
    kvr = [nc.dram_tensor(f"kvr{l}", [C, B, 2, 256, 256], BF,
                          addr_space="Shared") for l in range(L)]

    def ag(src_ap, dst_ap):
        nc.gpsimd.collective_compute(
            "AllGather", mybir.AluOpType.bypass,
            replica_groups=[list(range(C))],
            ins=[src_ap], outs=[dst_ap])

    with tile.TileContext(nc) as tc, ExitStack() as st:
        npool = st.enter_context(tc.tile_pool(name="npool", bufs=1))
        sbh = st.enter_context(tc.tile_pool(name="sbh", bufs=2))
        psA = st.enter_context(tc.tile_pool(name="psA", bufs=2, space="PSUM"))
        psB = st.enter_context(tc.tile_pool(name="psB", bufs=2, space="PSUM"))
        psM = st.enter_context(tc.tile_pool(name="psM", bufs=3, space="PSUM"))

        with ExitStack() as body:
            const = body.enter_context(tc.tile_pool(name="const", bufs=1))
            resid = body.enter_context(tc.tile_pool(name="resid", bufs=1))
            qpool = body.enter_context(tc.tile_pool(name="qpool", bufs=1))
            apool = body.enter_context(tc.tile_pool(name="apool", bufs=1))
            hpool = body.enter_context(tc.tile_pool(name="hpool", bufs=1))
            sb = body.enter_context(tc.tile_pool(name="sb", bufs=2))
            wbig = body.enter_context(tc.tile_pool(name="wbig", bufs=2))

            # constants
            ones_col = const.tile([128, 1], BF, tag="ones_col")
            nc.any.memset(ones_col[:], 1.0)
            ones_row = const.tile([1, 128], BF, tag="ones_row")
            nc.any.memset(ones_row[:], 1.0)
            eps_t = const.tile([1, 1], F32, tag="eps")
            nc.any.memset(eps_t[:], EPS)
            t_rmat = const.tile([128, 128], BF, tag="rmat")
            nc.sync.dma_start(out=t_rmat[:], in_=rmat[:])
            t_cos = const.tile([128, OWN], F32, tag="cos")
            nc.sync.dma_start(out=t_cos[:], in_=cosT[:])
            t_sin = const.tile([128, OWN], F32, tag="sin")
            nc.sync.dma_start(out=t_sin[:], in_=sinT[:])
            t_masks = [const.tile([128, 512], BF, tag=f"mask{m}",
                                  name=f"mask{m}") for m in range(16)]
            for m in range(16):
                nc.sync.dma_start(out=t_masks[m][:], in_=masks[m])

            # residual stream xT: 8 chunks [128, OWN] f32, resident
            x = [resid.tile([128, OWN], F32, tag=f"x{k}", name=f"x{k}")
                 for k in range(8)]
            for k in range(8):
                nc.sync.dma_start(out=x[k][:], in_=x0[k])

            # stage weight shards + first AG; remaining staging on scalar q
            nc.sync.dma_start(out=qkvo_st[0], in_=qkvo_sh[0])
            ag(qkvo_st[0], qkvo_g[0][:])
            nc.scalar.dma_start(out=gu_st[0], in_=gu_sh[0])
            nc.scalar.dma_start(out=wd_st[0], in_=wd_sh[0])
            nc.scalar.dma_start(out=qkvo_st[1], in_=qkvo_sh[1])
            nc.scalar.dma_start(out=gu_st[1], in_=gu_sh[1])
            nc.scalar.dma_start(out=wd_st[1], in_=wd_sh[1])

            def rmsnorm():
                """x -> n bf16 chunks (npool tags n0..n7, reused per call)."""
                ssq = psB.tile([1, OWN], F32, tag="psB")
                for k in range(8):
                    x2 = sb.tile([128, OWN], BF, tag="x2")
                    nc.scalar.activation(out=x2[:], in_=x[k][:], func=Square)
                    nc.tensor.matmul(out=ssq[:], lhsT=ones_col[:], rhs=x2[:],
                                     start=(k == 0), stop=(k == 7))
                rms = sbh.tile([1, OWN], F32, tag="rms", bufs=1)
                nc.scalar.activation(out=rms[:], in_=ssq[:], func=Sqrt,
                                     scale=1.0 / D, bias=eps_t[:])
                inv = sbh.tile([1, OWN], F32, tag="inv", bufs=1)
                nc.vector.reciprocal(out=inv[:], in_=rms[:])
                inv_bf = sbh.tile([1, OWN], BF, tag="invbf")
                nc.vector.tensor_copy(out=inv_bf[:], in_=inv[:])
                binv = psB.tile([128, OWN], F32, tag="psB")
                nc.tensor.matmul(out=binv[:], lhsT=ones_row[:], rhs=inv_bf[:],
                                 start=True, stop=True)
                n = [npool.tile([128, OWN], BF, tag=f"n{k}", name=f"n{k}")
                     for k in range(8)]
                for k in range(8):
                    nc.vector.tensor_tensor(out=n[k][:], in0=x[k][:],
                                            in1=binv[:], op=MULT)
                return n

            def rope(pm, y):
                """pm: psum [128, OWN] pre-rope -> bf16 tile y with rope."""
                yr = sb.tile([128, OWN], BF, tag="prerope")
                nc.vector.tensor_copy(out=yr[:], in_=pm[:])
                rot = psA.tile([128, OWN], F32, tag="psA")
                nc.tensor.matmul(out=rot[:], lhsT=t_rmat[:], rhs=yr[:],
                                 start=True, stop=True)
                tmp1 = sb.tile([128, OWN], F32, tag="ropet1", bufs=1)
                nc.vector.tensor_tensor(out=tmp1[:], in0=yr[:], in1=t_cos[:],
                                        op=MULT)
                tmp2 = sb.tile([128, OWN], F32, tag="ropet2", bufs=1)
                nc.vector.tensor_tensor(out=tmp2[:], in0=rot[:], in1=t_sin[:],
                                        op=MULT)
                nc.vector.tensor_tensor(out=y[:], in0=tmp1[:], in1=tmp2[:],
                                        op=ADD)
                return y

            for l in range(L):
                n = rmsnorm()
                # ---- k/v first so the kv AllGather starts early ----
                wkv = []
                for k in range(8):
                    wt = wbig.tile([128, 512], BF, tag=f"wbig{k}",
                                   name=f"wkv{k}")
                    nc.sync.dma_start(out=wt[:],
                                      in_=qkvo_g[l][k, :, 1024:1536])
                    wkv.append(wt)
                kr = [sb.tile([128, OWN], BF, tag=f"kr{mo}", name=f"kr{mo}")
                      for mo in range(2)]
                for mo in range(2):
                    pm = psM.tile([128, OWN], F32, tag="pmm")
                    for k in range(8):
                        nc.tensor.matmul(out=pm[:],
                                         lhsT=wkv[k][:, 128 * mo:128 * (mo + 1)],
                                         rhs=n[k][:], start=(k == 0),
                                         stop=(k == 7))
                    rope(pm, kr[mo])
                # v natural [own tok, 256]; lhsT = n col-slices
                for t in range(4):
                    pv = psM.tile([128, 256], F32, tag="pmm")
                    for k in range(8):
                        nc.tensor.matmul(out=pv[:],
                                         lhsT=n[k][:, 128 * t:128 * (t + 1)],
                                         rhs=wkv[k][:, 256:512],
                                         start=(k == 0), stop=(k == 7))
                    vt = sb.tile([128, 256], BF, tag="vnat")
                    nc.vector.tensor_copy(out=vt[:], in_=pv[:])
                    nc.sync.dma_start(
                        out=kvs[l][t // 2, 1, 128 * (t % 2):128 * (t % 2 + 1), :],
                        in_=vt[:])
                for b in range(B):
                    for mo in range(2):
                        nc.sync.dma_start(
                            out=kvs[l][b, 0, 128 * mo:128 * (mo + 1), :],
                            in_=kr[mo][:, 256 * b:256 * (b + 1)])
                ag(kvs[l][:], kvr[l][:])

                # ---- q (overlaps the kv AllGather) ----
                wqt = []
                for k in range(8):
                    wt = wbig.tile([128, 1024], BF, tag=f"wbig{k}",
                                   name=f"wq{k}")
                    nc.sync.dma_start(out=wt[:], in_=qkvo_g[l][k, :, 0:1024])
                    wqt.append(wt)
                qr = [qpool.tile([128, OWN], BF, tag=f"qr{mo}", name=f"qr{mo}")
                      for mo in range(8)]
                for mo in range(8):
                    pm = psM.tile([128, OWN], F32, tag="pmm")
                    for k in range(8):
                        nc.tensor.matmul(out=pm[:],
                                         lhsT=wqt[k][:, 128 * mo:128 * (mo + 1)],
                                         rhs=n[k][:], start=(k == 0),
                                         stop=(k == 7))
                    rope(pm, qr[mo])

                # queue the FFN weight AGs behind the kv AG
                ag(gu_st[l], gu_g[l][:])
                ag(wd_st[l], wd_g[l][:])
                if l == 0:
                    ag(qkvo_st[1], qkvo_g[1][:])

                # ---- attention (per batch: assemble k/v, run units) ----
                casm = [apool.tile([128, OWN], BF, tag=f"casm{k}",
                                   name=f"casm{k}") for k in range(8)]
                for b in range(B):
                    kT = [apool.tile([64, S], BF, tag=f"kt{g}", name=f"kt{g}")
                          for g in range(KVH)]
                    for g in range(KVH):
                        src = kvr[l][:, b, 0, 64 * g:64 * (g + 1), :]
                        nc.sync.dma_start(
                            out=kT[g][:, 0:1024].rearrange(
                                "p (r c) -> p r c", r=C),
                            in_=src[:, :, 0:128].transpose([1, 0, 2]))
                        for r in range(C):
                            nc.sync.dma_start(
                                out=kT[g][:, 128 * (NB - 1 - r):128 * (NB - r)],
                                in_=src[r, :, 128:256])
                    v4 = [apool.tile([128, 260], BF, tag=f"v4{j}",
                                     name=f"v4{j}") for j in range(NB)]
                    for j in range(NB):
                        r, i = (j, 0) if j < C else (NB - 1 - j, 1)
                        dst = v4[j][:].rearrange("p (g c) -> p g c", g=4)
                        nc.sync.dma_start(
                            out=dst[:, :, 0:64],
                            in_=kvr[l][r, b, 1, 128 * i:128 * (i + 1), :]
                                .rearrange("p (g c) -> p g c", g=4))
                        nc.any.memset(dst[:, :, 64:65], 1.0)

                    for g in range(KVH):
                        for t in range(2):      # t=0: low block, t=1: high
                            qp = sb.tile([64, 512], BF, tag="qpack")
                            for i in range(4):
                                h = 4 * g + i
                                mo, ro = divmod(h, 2)
                                nc.vector.tensor_copy(
                                    out=qp[:, 128 * i:128 * (i + 1)],
                                    in_=qr[mo][64 * ro:64 * (ro + 1),
                                               _colseg(b, t)])
                            ctx = psB.tile([65, 512], F32, tag="psB")
                            nj = 8 if t == 0 else 16
                            for j in range(nj):
                                sc = psA.tile([128, 512], F32, tag="psA")
                                nc.tensor.matmul(
                                    out=sc[:],
                                    lhsT=kT[g][:, 128 * j:128 * (j + 1)],
                                    rhs=qp[:], start=True, stop=True)
                                ex = sb.tile([128, 512], BF, tag="exp")
                                nc.scalar.activation(out=ex[:], in_=sc[:],
                                                     func=Exp)
                                if t == 0 or j >= 8:
                                    exm = sb.tile([128, 512], BF, tag="expm")
                                    m = t_masks[j if t == 0 else j]
                                    nc.vector.tensor_tensor(
                                        out=exm[:], in0=ex[:], in1=m[:],
                                        op=MULT)
                                    ex = exm
                                nc.tensor.matmul(
                                    out=ctx[:],
                                    lhsT=v4[j][:, 65 * g:65 * (g + 1)],
                                    rhs=ex[:], start=(j == 0),
                                    stop=(j == nj - 1))
                            rec = sb.tile([1, 512], F32, tag="rec")
                            nc.vector.reciprocal(out=rec[:], in_=ctx[64:65, :])
                            rec_bf = sb.tile([1, 512], BF, tag="recbf")
                            nc.vector.tensor_copy(out=rec_bf[:], in_=rec[:])
                            brec = psA.tile([64, 512], F32, tag="psA")
                            nc.tensor.matmul(out=brec[:],
                                             lhsT=ones_row[:1, 0:64],
                                             rhs=rec_bf[:], start=True,
                                             stop=True)
                            brec_s = sb.tile([64, 512], BF, tag="brecs")
                            nc.vector.tensor_copy(out=brec_s[:], in_=brec[:])
                            for i in range(4):
                                h = 4 * g + i
                                mo, ro = divmod(h, 2)
                                nc.vector.tensor_tensor(
                                    out=casm[mo][64 * ro:64 * (ro + 1),
                                                 _colseg(b, t)],
                                    in0=ctx[0:64, 128 * i:128 * (i + 1)],
                                    in1=brec_s[:, 128 * i:128 * (i + 1)],
                                    op=MULT)

                # ---- wo + residual ----
                wot = []
                for k in range(8):
                    wt = wbig.tile([128, 1024], BF, tag=f"wbig{k}",
                                   name=f"wo{k}")
                    nc.sync.dma_start(out=wt[:],
                                      in_=qkvo_g[l][k, :, 1536:2560])
                    wot.append(wt)
                for mo in range(8):
                    pm = psM.tile([128, OWN], F32, tag="pmm")
                    for k in range(8):
                        nc.tensor.matmul(out=pm[:],
                                         lhsT=wot[k][:, 128 * mo:128 * (mo + 1)],
                                         rhs=casm[k][:], start=(k == 0),
                                         stop=(k == 7))
                    nc.vector.tensor_tensor(out=x[mo][:], in0=x[mo][:],
                                            in1=pm[:], op=ADD)

                # ---- FFN ----
                n2 = rmsnorm()
                ht = [hpool.tile([128, OWN], BF, tag=f"h{mo}", name=f"h{mo}")
                      for mo in range(32)]
                for mb in range(8):
                    wgu = []
                    for k in range(8):
                        a = wbig.tile([128, 1024], BF, tag=f"wbig{k}",
                                      name=f"wgu{k}")
                        nc.sync.dma_start(
                            out=a[:],
                            in_=gu_g[l][k, :, 1024 * mb:1024 * (mb + 1)])
                        wgu.append(a)
                    for ms in range(4):
                        mo = 4 * mb + ms
                        pg = psM.tile([128, OWN], F32, tag="pmm")
                        for k in range(8):
                            nc.tensor.matmul(
                                out=pg[:],
                                lhsT=wgu[k][:, 128 * ms:128 * (ms + 1)],
                                rhs=n2[k][:], start=(k == 0), stop=(k == 7))
                        gs = sb.tile([128, OWN], BF, tag="gsilu")
                        nc.scalar.activation(out=gs[:], in_=pg[:], func=Silu)
                        pu = psM.tile([128, OWN], F32, tag="pmm")
                        for k in range(8):
                            nc.tensor.matmul(
                                out=pu[:],
                                lhsT=wgu[k][:, 512 + 128 * ms:512 + 128 * (ms + 1)],
                                rhs=n2[k][:], start=(k == 0), stop=(k == 7))
                        nc.vector.tensor_tensor(out=ht[mo][:], in0=pu[:],
                                                in1=gs[:], op=MULT)
                # down-proj: two output chunks per pass, stream wd tiles
                for mp in range(4):
                    pd0 = psM.tile([128, OWN], F32, tag="pmm")
                    pd1 = psM.tile([128, OWN], F32, tag="pmm")
                    for kk in range(32):
                        c_, j = divmod(kk, 4)
                        wt = wbig.tile([128, 256], BF, tag="wsm", bufs=4,
                                       name="wdt")
                        eng = nc.sync if kk % 2 == 0 else nc.scalar
                        eng.dma_start(
                            out=wt[:],
                            in_=wd_g[l][c_, :,
                                        1024 * j + 256 * mp:1024 * j + 256 * (mp + 1)])
                        nc.tensor.matmul(out=pd0[:], lhsT=wt[:, 0:128],
                                         rhs=ht[kk][:], start=(kk == 0),
                                         stop=(kk == 31))
                        nc.tensor.matmul(out=pd1[:], lhsT=wt[:, 128:256],
                                         rhs=ht[kk][:], start=(kk == 0),
                                         stop=(kk == 31))
                    nc.vector.tensor_tensor(out=x[2 * mp][:], in0=x[2 * mp][:],
                                            in1=pd0[:], op=ADD)
                    nc.vector.tensor_tensor(out=x[2 * mp + 1][:],
                                            in0=x[2 * mp + 1][:],
                                            in1=pd1[:], op=ADD)

            # ---- final norm -> nf (npool, survives body pools) ----
            nf = rmsnorm()
            for k in range(8):
                nc.sync.dma_start(out=nf_st[:, 512 * k:512 * (k + 1)],
                                  in_=nf[k][:])
            ag(nf_st[:], nf_g[:])

        # ---- vocab-sharded head: all tokens x our V/8 slice ----
        with ExitStack() as hd:
            hp = hd.enter_context(tc.tile_pool(name="hp", bufs=1))
            hw = hd.enter_context(tc.tile_pool(name="hw", bufs=2))
            whead = []
            for k in range(8):
                wt = hp.tile([128, VS], BF, tag=f"wh{k}", name=f"wh{k}")
                nc.sync.dma_start(out=wt[:], in_=woutc[128 * k:128 * (k + 1), :])
                whead.append(wt)
            for cp in range(C):
                nfo = hw.tile([128, 8 * OWN], BF, tag="nfo")
                nc.sync.dma_start(out=nfo[:], in_=nf_g[cp])
                for tb in range(4):
                    for vt in range(NVT):
                        vw = min(512, VS - 512 * vt)
                        ph = psM.tile([128, 512], F32, tag="pmm")
                        for k in range(8):
                            nc.tensor.matmul(
                                out=ph[:, :vw],
                                lhsT=nfo[:, 512 * k + 128 * tb:
                                         512 * k + 128 * (tb + 1)],
                                rhs=whead[k][:, 512 * vt:512 * vt + vw],
                                start=(k == 0), stop=(k == 7))
                        ot = hw.tile([128, 512], F16, tag="hout")
                        nc.vector.tensor_copy(out=ot[:, :vw], in_=ph[:, :vw])
                        nc.sync.dma_start(
                            out=logits[512 * cp + 128 * tb:
                                       512 * cp + 128 * (tb + 1),
                                       512 * vt:512 * vt + vw],
                            in_=ot[:, :vw])

    return nc


# ------------------------------------------------------------------ host ---

_NC_CACHE = {}


def _get_nc():
    if "nc" not in _NC_CACHE:
        nc = build_nc()
        nc.finalize()
        _NC_CACHE["nc"] = nc
    return _NC_CACHE["nc"]


def _host_prep(inputs):
    inv_freq = 1.0 / ROPE_BASE ** (np.arange(0, HD, 2, dtype=np.float32) / HD)
    t = np.arange(S, dtype=np.float32)
    freqs = t[:, None] * inv_freq[None, :]
    ang = np.concatenate([freqs, freqs], axis=-1)       # [S, 64]
    cos_full, sin_full = np.cos(ang), np.sin(ang)
    cosT2 = np.empty((128, S), np.float32)
    sinT2 = np.empty((128, S), np.float32)
    for p in range(128):
        d = p % 64
        cosT2[p] = cos_full[:, d]
        sinT2[p] = sin_full[:, d] * (-1.0 if d < 32 else 1.0)

    R = np.zeros((128, 128), np.float32)
    for blk in range(2):
        o = blk * 64
        for j in range(32):
            R[o + 32 + j, o + j] = 1.0
            R[o + j, o + 32 + j] = 1.0

    naw = np.asarray(inputs["norm_attn_w"], np.float32)
    nfw = np.asarray(inputs["norm_ff_w"], np.float32)
    emb = np.asarray(inputs["token_emb"], np.float32)
    wq_ = (np.asarray(inputs["wq"], np.float32) * naw[:, :, None] * SCALE
           ).astype(BF16)
    wk_ = (np.asarray(inputs["wk"], np.float32) * naw[:, :, None]).astype(BF16)
    wv_ = (np.asarray(inputs["wv"], np.float32) * naw[:, :, None]).astype(BF16)
    wo_ = np.asarray(inputs["wo"], np.float32).astype(BF16)
    wg_ = (np.asarray(inputs["w_gate"], np.float32) * nfw[:, :, None]
           ).astype(BF16)
    wu_ = (np.asarray(inputs["w_up"], np.float32) * nfw[:, :, None]
           ).astype(BF16)
    wd_ = np.asarray(inputs["w_down"], np.float32).astype(BF16)
    wout_ = (np.asarray(inputs["w_out"], np.float32)
             * np.asarray(inputs["norm_final_w"], np.float32)[:, None]
             ).astype(BF16)
    rmat_b = np.ascontiguousarray(R.astype(BF16))

    idx_full = np.asarray(inputs["in_idx"]).astype(np.int64)
    tri = (np.arange(128)[:, None] <= np.arange(128)[None, :]).astype(np.float32)
    tri4 = np.tile(tri, (1, 4))
    in_maps = []
    for c in range(C):
        blks = own_blocks(c)
        rs = slice(128 * c, 128 * (c + 1))
        # own-token ids in column order (b, tt): (0,b0),(0,b1),(1,b0),(1,b1)
        ids = np.concatenate([idx_full[b, bl * BS:(bl + 1) * BS]
                              for b in range(B) for bl in blks])
        x0 = np.ascontiguousarray(
            emb[ids].T.reshape(8, 128, OWN).astype(np.float32))
        pos = np.concatenate([np.arange(bl * BS, (bl + 1) * BS) for bl in blks])
        cosT = np.ascontiguousarray(
            np.concatenate([cosT2[:, pos], cosT2[:, pos]], axis=1))
        sinT = np.ascontiguousarray(
            np.concatenate([sinT2[:, pos], sinT2[:, pos]], axis=1))
        mk = np.zeros((16, 128, 512), np.float32)
        for t_, blk in enumerate(blks):
            for jj in range(8):
                j = jj if t_ == 0 else jj + 8
                if j < blk:
                    mk[8 * t_ + jj] = 1.0
                elif j == blk:
                    mk[8 * t_ + jj] = tri4
        # reorder high-block masks so t=1 uses t_masks[j] directly (j=8..15)
        qkvo = np.ascontiguousarray(np.concatenate(
            [wq_[:, rs, :], wk_[:, rs, :], wv_[:, rs, :], wo_[:, rs, :]],
            axis=2))
        gu = np.empty((L, 128, 8192), BF16)
        for mb in range(8):
            gu[:, :, 1024 * mb:1024 * mb + 512] = \
                wg_[:, rs, 512 * mb:512 * (mb + 1)]
            gu[:, :, 1024 * mb + 512:1024 * (mb + 1)] = \
                wu_[:, rs, 512 * mb:512 * (mb + 1)]
        wdsh = np.ascontiguousarray(
            wd_[:, 512 * c:512 * (c + 1), :]
            .reshape(L, 4, 128, D).transpose(0, 2, 1, 3).reshape(L, 128, 4096))
        in_maps.append({
            "x0": x0,
            "cosT": cosT,
            "sinT": sinT,
            "rmat": rmat_b,
            "masks": np.ascontiguousarray(mk.astype(BF16)),
            "qkvo_sh": qkvo,
            "gu_sh": np.ascontiguousarray(gu),
            "wd_sh": wdsh,
            "woutc": np.ascontiguousarray(wout_[:, VS * c:VS * (c + 1)]),
        })
    return in_maps


def _assemble(results):
    out = np.empty((B, S, V), np.float32)
    for c in range(C):          # vocab-shard owner
        lg = np.asarray(results[c]["logits"]).astype(np.float32)
        for cp in range(C):     # token owner
            blks = own_blocks(cp)
            for b in range(B):
                for tt in range(2):
                    r0 = cp * 512 + 128 * (2 * b + tt)
                    out[b, blks[tt] * BS:(blks[tt] + 1) * BS,
                        VS * c:VS * (c + 1)] = lg[r0:r0 + 128]
    return out


def run(inputs, trace=False, trace_cores=None):
    nc = _get_nc()
    in_maps = _host_prep(inputs)
    res = run_bass_kernel_spmd(nc, in_maps, list(range(C)), trace=trace,
                               trace_cores=trace_cores)
    return _assemble(res.results), res


def kernel(**inputs):
    out, _ = run(inputs)
    return out


# revision 19
# speedup vs baseline: 1.0288x; 1.0239x over previous
"""Self-contained Trainium2 Bass kernel for the 2-layer Llama3 model.

Sharding: token-sharded compute over 8 cores (core c owns token blocks
{c, 15-c} of each batch, 512 tokens/core), with *minimal host->device
input bytes*: every weight is shipped exactly once, row-sharded by its
contraction dim (1/8 per core), and AllGathered on-device over the
intra-chip links, overlapped with compute. Embedding rows are gathered
on host (2MB/core instead of the 131MB fp32 table). The vocab head is
vocab-sharded: each core keeps its [D, V/8] slice of w_out, final
hidden states are AllGathered (8MB), and each core computes logits for
ALL tokens x its vocab slice, emitted as fp16 (host casts to f32).

Per-core input ~20MB (vs ~257MB replicated), output 32MB fp16.

Device layouts: activations transposed [feature, token] in 128-part
chunks; scores computed transposed [sk, sq]; softmax denominator via a
ones-augmented column on v; RoPE via a +-1 rotation matrix on the PE.
SPMD-uniform program: per-core causality lives in mask *data*.
"""
from contextlib import ExitStack

import numpy as np
import ml_dtypes

import concourse.bass as bass
from concourse.bacc import Bacc
import concourse.mybir as mybir
import concourse.tile as tile
from concourse.bass_utils import run_bass_kernel_spmd

BF16 = ml_dtypes.bfloat16
F32 = mybir.dt.float32
F16 = mybir.dt.float16
BF = mybir.dt.bfloat16

V, D, H, KVH, HD, F, L, B, S = 32000, 1024, 16, 4, 64, 4096, 2, 2, 2048
ROPE_BASE = 500000.0
EPS = 1e-5
SCALE = 1.0 / HD ** 0.5
C = 8            # cores
NB = 16          # 128-token blocks per batch
BS = 128         # block size
OWN = 512        # own tokens per core (2 blocks x 2 batches)
VS = V // C      # vocab shard per core (4000)
NVT = (VS + 511) // 512   # 8 vocab tiles (last is 416 wide)

Exp = mybir.ActivationFunctionType.Exp
Silu = mybir.ActivationFunctionType.Silu
Ln = mybir.ActivationFunctionType.Ln
Copy = mybir.ActivationFunctionType.Copy
Square = mybir.ActivationFunctionType.Square
MULT = mybir.AluOpType.mult
ADD = mybir.AluOpType.add


def own_blocks(c):
    return [c, NB - 1 - c]


def _colseg(b, t):
    """Own-token column range for (batch b, tile t in {0=low,1=high})."""
    return slice(256 * b + 128 * t, 256 * b + 128 * (t + 1))


# ---------------------------------------------------------------- device ---

def build_nc():
    nc = Bacc()

    x0 = nc.dram_tensor("x0", [8, 128, OWN], F32, kind="ExternalInput")
    cosT = nc.dram_tensor("cosT", [128, OWN], F32, kind="ExternalInput")
    sinT = nc.dram_tensor("sinT", [128, OWN], F32, kind="ExternalInput")
    rmat = nc.dram_tensor("rmat", [128, 128], BF, kind="ExternalInput")
    masks = nc.dram_tensor("masks", [16, 128, 512], BF, kind="ExternalInput")
    # weight shards, row-sharded by contraction dim (1/8 per core):
    # kvw_sh cols: wk 0:256 | wv 256:512.  qo_sh cols: wq 0:1024 | wo 1024:2048
    kvw_sh = nc.dram_tensor("kvw_sh", [L, 128, 512], BF, kind="ExternalInput")
    qo_sh = nc.dram_tensor("qo_sh", [L, 128, 2048], BF, kind="ExternalInput")
    # gu_sh cols: 8 groups of [wg 512 | wu 512]
    gu_sh = nc.dram_tensor("gu_sh", [L, 128, 8192], BF, kind="ExternalInput")
    # wd_sh cols: 4 groups of 1024 (F-chunks 4c..4c+3, each [128, D])
    wd_sh = nc.dram_tensor("wd_sh", [L, 128, 4096], BF, kind="ExternalInput")
    woutc = nc.dram_tensor("woutc", [D, VS], BF, kind="ExternalInput")
    logits = nc.dram_tensor("logits", [C * OWN, VS], F16, kind="ExternalOutput")

    # collective staging (internal) and gathered (Shared) buffers
    kvw_st = nc.dram_tensor("kvw_st", [L, 128, 512], BF)
    qo_st = nc.dram_tensor("qo_st", [L, 128, 2048], BF)
    gu_st = nc.dram_tensor("gu_st", [L, 128, 8192], BF)
    wd_st = nc.dram_tensor("wd_st", [L, 128, 4096], BF)
    nf_st = nc.dram_tensor("nf_st", [128, 8 * OWN], BF)
    kvw_g = [nc.dram_tensor(f"kvwg{l}", [C, 128, 512], BF,
                            addr_space="Shared") for l in range(L)]
    qo_g = [nc.dram_tensor(f"qog{l}", [C, 128, 2048], BF,
                           addr_space="Shared") for l in range(L)]
    gu_g = [nc.dram_tensor(f"gug{l}", [C, 128, 8192], BF,
                           addr_space="Shared") for l in range(L)]
    wd_g = [nc.dram_tensor(f"wdg{l}", [C, 128, 4096], BF,
                           addr_space="Shared") for l in range(L)]
    nf_g = nc.dram_tensor("nfg", [C, 128, 8 * OWN], BF, addr_space="Shared")
    kvs = [nc.dram_tensor(f"kvs{l}", [B, 2, 256, 256], BF) for l in range(L)]
    kvr = [nc.dram_tensor(f"kvr{l}", [C, B, 2, 256, 256], BF,
                          addr_space="Shared") for l in range(L)]

    def ag(src_ap, dst_ap):
        nc.gpsimd.collective_compute(
            "AllGather", mybir.AluOpType.bypass,
            replica_groups=[list(range(C))],
            ins=[src_ap], outs=[dst_ap])

    with tile.TileContext(nc) as tc, ExitStack() as st:
        npool = st.enter_context(tc.tile_pool(name="npool", bufs=1))
        sbh = st.enter_context(tc.tile_pool(name="sbh", bufs=2))
        psA = st.enter_context(tc.tile_pool(name="psA", bufs=2, space="PSUM"))
        psB = st.enter_context(tc.tile_pool(name="psB", bufs=2, space="PSUM"))
        psM = st.enter_context(tc.tile_pool(name="psM", bufs=3, space="PSUM"))

        with ExitStack() as body:
            const = body.enter_context(tc.tile_pool(name="const", bufs=1))
            resid = body.enter_context(tc.tile_pool(name="resid", bufs=1))
            qpool = body.enter_context(tc.tile_pool(name="qpool", bufs=1))
            apool = body.enter_context(tc.tile_pool(name="apool", bufs=1))
            hpool = body.enter_context(tc.tile_pool(name="hpool", bufs=1))
            sb = body.enter_context(tc.tile_pool(name="sb", bufs=2))
            wbig = body.enter_context(tc.tile_pool(name="wbig", bufs=2))

            # residual stream xT first: 8 chunks [128, OWN] f32, resident
            x = [resid.tile([128, OWN], F32, tag=f"x{k}", name=f"x{k}")
                 for k in range(8)]
            for k in range(8):
                nc.sync.dma_start(out=x[k][:], in_=x0[k])

            # stage weight shards + first AGs; staging copies on scalar q
            nc.sync.dma_start(out=kvw_st[0], in_=kvw_sh[0])
            ag(kvw_st[0], kvw_g[0][:])
            nc.sync.dma_start(out=qo_st[0], in_=qo_sh[0])
            ag(qo_st[0], qo_g[0][:])
            nc.scalar.dma_start(out=gu_st[0], in_=gu_sh[0])
            nc.scalar.dma_start(out=wd_st[0], in_=wd_sh[0])
            nc.scalar.dma_start(out=kvw_st[1], in_=kvw_sh[1])
            nc.scalar.dma_start(out=qo_st[1], in_=qo_sh[1])
            nc.scalar.dma_start(out=gu_st[1], in_=gu_sh[1])
            nc.scalar.dma_start(out=wd_st[1], in_=wd_sh[1])

            # constants
            ones_col = const.tile([128, 1], BF, tag="ones_col")
            nc.any.memset(ones_col[:], 1.0)
            ones_row = const.tile([1, 128], BF, tag="ones_row")
            nc.any.memset(ones_row[:], 1.0)
            eps_t = const.tile([1, 1], F32, tag="eps")
            nc.any.memset(eps_t[:], EPS)
            t_rmat = const.tile([128, 128], BF, tag="rmat")
            nc.sync.dma_start(out=t_rmat[:], in_=rmat[:])
            t_cos = const.tile([128, OWN], F32, tag="cos")
            nc.sync.dma_start(out=t_cos[:], in_=cosT[:])
            t_sin = const.tile([128, OWN], F32, tag="sin")
            nc.sync.dma_start(out=t_sin[:], in_=sinT[:])
            t_masks = [const.tile([128, 512], BF, tag=f"mask{m}",
                                  name=f"mask{m}") for m in range(16)]
            for m in range(16):
                nc.scalar.dma_start(out=t_masks[m][:], in_=masks[m])

            def rmsnorm():
                """x -> n bf16 chunks (npool tags n0..n7, reused per call)."""
                ssq = psB.tile([1, OWN], F32, tag="psB")
                for k in range(8):
                    x2 = sb.tile([128, OWN], BF, tag="x2")
                    nc.scalar.activation(out=x2[:], in_=x[k][:], func=Square)
                    nc.tensor.matmul(out=ssq[:], lhsT=ones_col[:], rhs=x2[:],
                                     start=(k == 0), stop=(k == 7))
                # 1/sqrt(m+eps) = exp(-0.5*ln(m+eps)); both on ACT (fast),
                # avoids the 4.3us single-lane DVE reciprocal
                lnm = sbh.tile([1, OWN], F32, tag="lnm", bufs=1)
                nc.scalar.activation(out=lnm[:], in_=ssq[:], func=Ln,
                                     scale=1.0 / D, bias=eps_t[:])
                inv_bf = sbh.tile([1, OWN], BF, tag="invbf")
                nc.scalar.activation(out=inv_bf[:], in_=lnm[:], func=Exp,
                                     scale=-0.5)
                binv = psB.tile([128, OWN], F32, tag="psB")
                nc.tensor.matmul(out=binv[:], lhsT=ones_row[:], rhs=inv_bf[:],
                                 start=True, stop=True)
                n = [npool.tile([128, OWN], BF, tag=f"n{k}", name=f"n{k}")
                     for k in range(8)]
                for k in range(8):
                    nc.vector.tensor_tensor(out=n[k][:], in0=x[k][:],
                                            in1=binv[:], op=MULT)
                return n

            def rope(pm, y):
                """pm: psum [128, OWN] pre-rope -> bf16 tile y with rope."""
                yr = sb.tile([128, OWN], BF, tag="prerope")
                nc.vector.tensor_copy(out=yr[:], in_=pm[:])
                rot = psA.tile([128, OWN], F32, tag="psA")
                nc.tensor.matmul(out=rot[:], lhsT=t_rmat[:], rhs=yr[:],
                                 start=True, stop=True)
                tmp1 = sb.tile([128, OWN], F32, tag="ropet1", bufs=1)
                nc.vector.tensor_tensor(out=tmp1[:], in0=yr[:], in1=t_cos[:],
                                        op=MULT)
                tmp2 = sb.tile([128, OWN], F32, tag="ropet2", bufs=1)
                nc.vector.tensor_tensor(out=tmp2[:], in0=rot[:], in1=t_sin[:],
                                        op=MULT)
                nc.vector.tensor_tensor(out=y[:], in0=tmp1[:], in1=tmp2[:],
                                        op=ADD)
                return y

            for l in range(L):
                n = rmsnorm()
                # ---- k/v first so the kv AllGather starts early ----
                wkv = []
                for k in range(8):
                    wt = wbig.tile([128, 512], BF, tag=f"wbig{k}",
                                   name=f"wkv{k}")
                    nc.sync.dma_start(out=wt[:], in_=kvw_g[l][k])
                    wkv.append(wt)
                kr = [sb.tile([128, OWN], BF, tag=f"kr{mo}", name=f"kr{mo}")
                      for mo in range(2)]
                for mo in range(2):
                    pm = psM.tile([128, OWN], F32, tag="pmm")
                    for k in range(8):
                        nc.tensor.matmul(out=pm[:],
                                         lhsT=wkv[k][:, 128 * mo:128 * (mo + 1)],
                                         rhs=n[k][:], start=(k == 0),
                                         stop=(k == 7))
                    rope(pm, kr[mo])
                # v natural [own tok, 256]; lhsT = n col-slices
                for t in range(4):
                    pv = psM.tile([128, 256], F32, tag="pmm")
                    for k in range(8):
                        nc.tensor.matmul(out=pv[:],
                                         lhsT=n[k][:, 128 * t:128 * (t + 1)],
                                         rhs=wkv[k][:, 256:512],
                                         start=(k == 0), stop=(k == 7))
                    vt = sb.tile([128, 256], BF, tag="vnat")
                    nc.vector.tensor_copy(out=vt[:], in_=pv[:])
                    nc.sync.dma_start(
                        out=kvs[l][t // 2, 1, 128 * (t % 2):128 * (t % 2 + 1), :],
                        in_=vt[:])
                for b in range(B):
                    for mo in range(2):
                        nc.sync.dma_start(
                            out=kvs[l][b, 0, 128 * mo:128 * (mo + 1), :],
                            in_=kr[mo][:, 256 * b:256 * (b + 1)])
                ag(kvs[l][:], kvr[l][:])

                # ---- q (overlaps the kv AllGather) ----
                wqt = []
                for k in range(8):
                    wt = wbig.tile([128, 1024], BF, tag=f"wbig{k}",
                                   name=f"wq{k}")
                    nc.sync.dma_start(out=wt[:], in_=qo_g[l][k, :, 0:1024])
                    wqt.append(wt)
                qr = [qpool.tile([128, OWN], BF, tag=f"qr{mo}", name=f"qr{mo}")
                      for mo in range(8)]
                for mo in range(8):
                    pm = psM.tile([128, OWN], F32, tag="pmm")
                    for k in range(8):
                        nc.tensor.matmul(out=pm[:],
                                         lhsT=wqt[k][:, 128 * mo:128 * (mo + 1)],
                                         rhs=n[k][:], start=(k == 0),
                                         stop=(k == 7))
                    rope(pm, qr[mo])

                # queue the FFN weight AGs behind the kv AG
                ag(gu_st[l], gu_g[l][:])
                ag(wd_st[l], wd_g[l][:])
                if l == 0:
                    ag(kvw_st[1], kvw_g[1][:])
                    ag(qo_st[1], qo_g[1][:])

                # ---- attention (per batch: assemble k/v, run units) ----
                casm = [apool.tile([128, OWN], BF, tag=f"casm{k}",
                                   name=f"casm{k}") for k in range(8)]
                for b in range(B):
                    kT = [apool.tile([64, S], BF, tag=f"kt{g}", name=f"kt{g}")
                          for g in range(KVH)]
                    for g in range(KVH):
                        src = kvr[l][:, b, 0, 64 * g:64 * (g + 1), :]
                        nc.sync.dma_start(
                            out=kT[g][:, 0:1024].rearrange(
                                "p (r c) -> p r c", r=C),
                            in_=src[:, :, 0:128].transpose([1, 0, 2]))
                        for r in range(C):
                            nc.sync.dma_start(
                                out=kT[g][:, 128 * (NB - 1 - r):128 * (NB - r)],
                                in_=src[r, :, 128:256])
                    v4 = [apool.tile([128, 260], BF, tag=f"v4{j}",
                                     name=f"v4{j}") for j in range(NB)]
                    for j in range(NB):
                        r, i = (j, 0) if j < C else (NB - 1 - j, 1)
                        dst = v4[j][:].rearrange("p (g c) -> p g c", g=4)
                        nc.sync.dma_start(
                            out=dst[:, :, 0:64],
                            in_=kvr[l][r, b, 1, 128 * i:128 * (i + 1), :]
                                .rearrange("p (g c) -> p g c", g=4))
                        nc.any.memset(dst[:, :, 64:65], 1.0)

                    for g in range(KVH):
                        for t in range(2):      # t=0: low block, t=1: high
                            qp = sb.tile([64, 512], BF, tag="qpack")
                            for i in range(4):
                                h = 4 * g + i
                                mo, ro = divmod(h, 2)
                                nc.vector.tensor_copy(
                                    out=qp[:, 128 * i:128 * (i + 1)],
                                    in_=qr[mo][64 * ro:64 * (ro + 1),
                                               _colseg(b, t)])
                            ctx = psB.tile([65, 512], F32, tag="psB")
                            nj = 8 if t == 0 else 16
                            for j in range(nj):
                                sc = psA.tile([128, 512], F32, tag="psA")
                                nc.tensor.matmul(
                                    out=sc[:],
                                    lhsT=kT[g][:, 128 * j:128 * (j + 1)],
                                    rhs=qp[:], start=True, stop=True)
                                ex = sb.tile([128, 512], BF, tag="exp")
                                nc.scalar.activation(out=ex[:], in_=sc[:],
                                                     func=Exp)
                                if t == 0 or j >= 8:
                                    exm = sb.tile([128, 512], BF, tag="expm")
                                    m = t_masks[j if t == 0 else j]
                                    nc.vector.tensor_tensor(
                                        out=exm[:], in0=ex[:], in1=m[:],
                                        op=MULT)
                                    ex = exm
                                nc.tensor.matmul(
                                    out=ctx[:],
                                    lhsT=v4[j][:, 65 * g:65 * (g + 1)],
                                    rhs=ex[:], start=(j == 0),
                                    stop=(j == nj - 1))
                            lnd = sb.tile([1, 512], F32, tag="lnd")
                            nc.scalar.activation(out=lnd[:],
                                                 in_=ctx[64:65, :], func=Ln)
                            rec_bf = sb.tile([1, 512], BF, tag="recbf")
                            nc.scalar.activation(out=rec_bf[:], in_=lnd[:],
                                                 func=Exp, scale=-1.0)
                            brec = psA.tile([64, 512], F32, tag="psA")
                            nc.tensor.matmul(out=brec[:],
                                             lhsT=ones_row[:1, 0:64],
                                             rhs=rec_bf[:], start=True,
                                             stop=True)
                            brec_s = sb.tile([64, 512], BF, tag="brecs")
                            nc.vector.tensor_copy(out=brec_s[:], in_=brec[:])
                            for i in range(4):
                                h = 4 * g + i
                                mo, ro = divmod(h, 2)
                                nc.vector.tensor_tensor(
                                    out=casm[mo][64 * ro:64 * (ro + 1),
                                                 _colseg(b, t)],
                                    in0=ctx[0:64, 128 * i:128 * (i + 1)],
                                    in1=brec_s[:, 128 * i:128 * (i + 1)],
                                    op=MULT)

                # ---- wo + residual ----
                wot = []
                for k in range(8):
                    wt = wbig.tile([128, 1024], BF, tag=f"wbig{k}",
                                   name=f"wo{k}")
                    nc.sync.dma_start(out=wt[:],
                                      in_=qo_g[l][k, :, 1024:2048])
                    wot.append(wt)
                for mo in range(8):
                    pm = psM.tile([128, OWN], F32, tag="pmm")
                    for k in range(8):
                        nc.tensor.matmul(out=pm[:],
                                         lhsT=wot[k][:, 128 * mo:128 * (mo + 1)],
                                         rhs=casm[k][:], start=(k == 0),
                                         stop=(k == 7))
                    nc.vector.tensor_tensor(out=x[mo][:], in0=x[mo][:],
                                            in1=pm[:], op=ADD)

                # ---- FFN ----
                n2 = rmsnorm()
                ht = [hpool.tile([128, OWN], BF, tag=f"h{mo}", name=f"h{mo}")
                      for mo in range(32)]
                for mb in range(8):
                    wgu = []
                    for k in range(8):
                        a = wbig.tile([128, 1024], BF, tag=f"wbig{k}",
                                      name=f"wgu{k}")
                        nc.sync.dma_start(
                            out=a[:],
                            in_=gu_g[l][k, :, 1024 * mb:1024 * (mb + 1)])
                        wgu.append(a)
                    for ms in range(4):
                        mo = 4 * mb + ms
                        pg = psM.tile([128, OWN], F32, tag="pmm")
                        for k in range(8):
                            nc.tensor.matmul(
                                out=pg[:],
                                lhsT=wgu[k][:, 128 * ms:128 * (ms + 1)],
                                rhs=n2[k][:], start=(k == 0), stop=(k == 7))
                        gs = sb.tile([128, OWN], BF, tag="gsilu")
                        nc.scalar.activation(out=gs[:], in_=pg[:], func=Silu)
                        pu = psM.tile([128, OWN], F32, tag="pmm")
                        for k in range(8):
                            nc.tensor.matmul(
                                out=pu[:],
                                lhsT=wgu[k][:, 512 + 128 * ms:512 + 128 * (ms + 1)],
                                rhs=n2[k][:], start=(k == 0), stop=(k == 7))
                        nc.vector.tensor_tensor(out=ht[mo][:], in0=pu[:],
                                                in1=gs[:], op=MULT)
                # down-proj: two output chunks per pass, stream wd tiles
                for mp in range(4):
                    pd0 = psM.tile([128, OWN], F32, tag="pmm")
                    pd1 = psM.tile([128, OWN], F32, tag="pmm")
                    for kk in range(32):
                        c_, j = divmod(kk, 4)
                        wt = wbig.tile([128, 256], BF, tag="wsm", bufs=4,
                                       name="wdt")
                        eng = nc.sync if kk % 2 == 0 else nc.scalar
                        eng.dma_start(
                            out=wt[:],
                            in_=wd_g[l][c_, :,
                                        1024 * j + 256 * mp:1024 * j + 256 * (mp + 1)])
                        nc.tensor.matmul(out=pd0[:], lhsT=wt[:, 0:128],
                                         rhs=ht[kk][:], start=(kk == 0),
                                         stop=(kk == 31))
                        nc.tensor.matmul(out=pd1[:], lhsT=wt[:, 128:256],
                                         rhs=ht[kk][:], start=(kk == 0),
                                         stop=(kk == 31))
                    nc.vector.tensor_tensor(out=x[2 * mp][:], in0=x[2 * mp][:],
                                            in1=pd0[:], op=ADD)
                    nc.vector.tensor_tensor(out=x[2 * mp + 1][:],
                                            in0=x[2 * mp + 1][:],
                                            in1=pd1[:], op=ADD)

            # ---- final norm -> nf (npool, survives body pools) ----
            nf = rmsnorm()
            for k in range(8):
                nc.sync.dma_start(out=nf_st[:, 512 * k:512 * (k + 1)],
                                  in_=nf[k][:])
            ag(nf_st[:], nf_g[:])

        # ---- vocab-sharded head: all tokens x our V/8 slice ----
        with ExitStack() as hd:
            hp = hd.enter_context(tc.tile_pool(name="hp", bufs=1))
            hw = hd.enter_context(tc.tile_pool(name="hw", bufs=2))
            whead = []
            for k in range(8):
                wt = hp.tile([128, VS], BF, tag=f"wh{k}", name=f"wh{k}")
                nc.sync.dma_start(out=wt[:], in_=woutc[128 * k:128 * (k + 1), :])
                whead.append(wt)
            for cp in range(C):
                nfo = hw.tile([128, 8 * OWN], BF, tag="nfo")
                nc.sync.dma_start(out=nfo[:], in_=nf_g[cp])
                for tb in range(4):
                    for vt in range(NVT):
                        vw = min(512, VS - 512 * vt)
                        ph = psM.tile([128, 512], F32, tag="pmm")
                        for k in range(8):
                            nc.tensor.matmul(
                                out=ph[:, :vw],
                                lhsT=nfo[:, 512 * k + 128 * tb:
                                         512 * k + 128 * (tb + 1)],
                                rhs=whead[k][:, 512 * vt:512 * vt + vw],
                                start=(k == 0), stop=(k == 7))
                        ot = hw.tile([128, 512], F16, tag="hout")
                        if vt % 2 == 0:
                            nc.vector.tensor_copy(out=ot[:, :vw],
                                                  in_=ph[:, :vw])
                        else:
                            nc.scalar.activation(out=ot[:, :vw],
                                                 in_=ph[:, :vw], func=Copy)
                        nc.sync.dma_start(
                            out=logits[512 * cp + 128 * tb:
                                       512 * cp + 128 * (tb + 1),
                                       512 * vt:512 * vt + vw],
                            in_=ot[:, :vw])

    return nc


# ------------------------------------------------------------------ host ---

_NC_CACHE = {}


def _get_nc():
    if "nc" not in _NC_CACHE:
        nc = build_nc()
        nc.finalize()
        _NC_CACHE["nc"] = nc
    return _NC_CACHE["nc"]


def _host_prep(inputs):
    inv_freq = 1.0 / ROPE_BASE ** (np.arange(0, HD, 2, dtype=np.float32) / HD)
    t = np.arange(S, dtype=np.float32)
    freqs = t[:, None] * inv_freq[None, :]
    ang = np.concatenate([freqs, freqs], axis=-1)       # [S, 64]
    cos_full, sin_full = np.cos(ang), np.sin(ang)
    cosT2 = np.empty((128, S), np.float32)
    sinT2 = np.empty((128, S), np.float32)
    for p in range(128):
        d = p % 64
        cosT2[p] = cos_full[:, d]
        sinT2[p] = sin_full[:, d] * (-1.0 if d < 32 else 1.0)

    R = np.zeros((128, 128), np.float32)
    for blk in range(2):
        o = blk * 64
        for j in range(32):
            R[o + 32 + j, o + j] = 1.0
            R[o + j, o + 32 + j] = 1.0

    naw = np.asarray(inputs["norm_attn_w"], np.float32)
    nfw = np.asarray(inputs["norm_ff_w"], np.float32)
    emb = np.asarray(inputs["token_emb"], np.float32)
    wq_ = (np.asarray(inputs["wq"], np.float32) * naw[:, :, None] * SCALE
           ).astype(BF16)
    wk_ = (np.asarray(inputs["wk"], np.float32) * naw[:, :, None]).astype(BF16)
    wv_ = (np.asarray(inputs["wv"], np.float32) * naw[:, :, None]).astype(BF16)
    wo_ = np.asarray(inputs["wo"], np.float32).astype(BF16)
    wg_ = (np.asarray(inputs["w_gate"], np.float32) * nfw[:, :, None]
           ).astype(BF16)
    wu_ = (np.asarray(inputs["w_up"], np.float32) * nfw[:, :, None]
           ).astype(BF16)
    wd_ = np.asarray(inputs["w_down"], np.float32).astype(BF16)
    wout_ = (np.asarray(inputs["w_out"], np.float32)
             * np.asarray(inputs["norm_final_w"], np.float32)[:, None]
             ).astype(BF16)
    rmat_b = np.ascontiguousarray(R.astype(BF16))

    idx_full = np.asarray(inputs["in_idx"]).astype(np.int64)
    tri = (np.arange(128)[:, None] <= np.arange(128)[None, :]).astype(np.float32)
    tri4 = np.tile(tri, (1, 4))
    in_maps = []
    for c in range(C):
        blks = own_blocks(c)
        rs = slice(128 * c, 128 * (c + 1))
        # own-token ids in column order (b, tt): (0,b0),(0,b1),(1,b0),(1,b1)
        ids = np.concatenate([idx_full[b, bl * BS:(bl + 1) * BS]
                              for b in range(B) for bl in blks])
        x0 = np.ascontiguousarray(
            emb[ids].T.reshape(8, 128, OWN).astype(np.float32))
        pos = np.concatenate([np.arange(bl * BS, (bl + 1) * BS) for bl in blks])
        cosT = np.ascontiguousarray(
            np.concatenate([cosT2[:, pos], cosT2[:, pos]], axis=1))
        sinT = np.ascontiguousarray(
            np.concatenate([sinT2[:, pos], sinT2[:, pos]], axis=1))
        mk = np.zeros((16, 128, 512), np.float32)
        for t_, blk in enumerate(blks):
            for jj in range(8):
                j = jj if t_ == 0 else jj + 8
                if j < blk:
                    mk[8 * t_ + jj] = 1.0
                elif j == blk:
                    mk[8 * t_ + jj] = tri4
        kvw = np.ascontiguousarray(np.concatenate(
            [wk_[:, rs, :], wv_[:, rs, :]], axis=2))
        qo = np.ascontiguousarray(np.concatenate(
            [wq_[:, rs, :], wo_[:, rs, :]], axis=2))
        gu = np.empty((L, 128, 8192), BF16)
        for mb in range(8):
            gu[:, :, 1024 * mb:1024 * mb + 512] = \
                wg_[:, rs, 512 * mb:512 * (mb + 1)]
            gu[:, :, 1024 * mb + 512:1024 * (mb + 1)] = \
                wu_[:, rs, 512 * mb:512 * (mb + 1)]
        wdsh = np.ascontiguousarray(
            wd_[:, 512 * c:512 * (c + 1), :]
            .reshape(L, 4, 128, D).transpose(0, 2, 1, 3).reshape(L, 128, 4096))
        in_maps.append({
            "x0": x0,
            "cosT": cosT,
            "sinT": sinT,
            "rmat": rmat_b,
            "masks": np.ascontiguousarray(mk.astype(BF16)),
            "kvw_sh": kvw,
            "qo_sh": qo,
            "gu_sh": np.ascontiguousarray(gu),
            "wd_sh": wdsh,
            "woutc": np.ascontiguousarray(wout_[:, VS * c:VS * (c + 1)]),
        })
    return in_maps


def _assemble(results):
    out = np.empty((B, S, V), np.float32)
    for c in range(C):          # vocab-shard owner
        lg = np.asarray(results[c]["logits"]).astype(np.float32)
        for cp in range(C):     # token owner
            blks = own_blocks(cp)
            for b in range(B):
                for tt in range(2):
                    r0 = cp * 512 + 128 * (2 * b + tt)
                    out[b, blks[tt] * BS:(blks[tt] + 1) * BS,
                        VS * c:VS * (c + 1)] = lg[r0:r0 + 128]
    return out


def run(inputs, trace=False, trace_cores=None):
    nc = _get_nc()
    in_maps = _host_prep(inputs)
    res = run_bass_kernel_spmd(nc, in_maps, list(range(C)), trace=trace,
                               trace_cores=trace_cores)
    return _assemble(res.results), res


def kernel(**inputs):
    out, _ = run(inputs)
    return out
